# revision 15
# baseline (speedup 1.0000x reference)
"""CRCDLoss Trainium2 kernel (8-core SPMD, Bass/Tile), v3.

Core strategy: replace the reference's per-(b,k) gather (~1.07 GB) with a
dense score matmul S[b, n] = v_b . mem_n over the full memory banks,
sharded along n across the 8 cores. The multiplicity counts
cnt[b, n] = #{k : idx_all[b,k] == n} are folded INTO THE EXPONENT:
one DoubleRow fp8 matmul per 512-col window computes both sides' scores
(PSUM rows 0:64 = v_s.mem2, 64:128 = v_t.mem1) and a second plain
identity matmul accumulates L[b, n] = ln(cnt)/escale_b into the same
PSUM bank, so that the activation exp(escale*(S+L)) = cnt * e directly.
The activation's free accum_out then yields the moment M1 = sum cnt*e
with no vector-engine pass at all. M2 is estimated from a stride-16
subsample of (cnt*e)^2 rescaled by the host ratio sum(cnt)/sum(cnt^2)
(valid since cnt and e are independent), entering only a tiny 2nd-order
series term. The Z normalizer is eliminated algebraically via the
2-term log series; the host combines per-core sums in float64.
Positives use host-gathered f32 rows against the normalized v.

Per-core HBM traffic: banks fused fp8 3.28MB + L fp8 1.64MB + embed
fp8 0.6MB => ~5.6MB (~16us at 358 GB/s), vs 9.8MB bf16 in v1.
"""

import sys

import numpy as np

try:
    import concourse.bass as bass  # noqa: F401
except ImportError:
    sys.path.insert(0, "/opt/trn_rl_repo")

import concourse.bacc as bacc
import concourse.bass as bass  # noqa: F811
import concourse.mybir as mybir
import concourse.tile as tile
from concourse.bass_utils import run_bass_kernel_spmd

import ml_dtypes

# ---- problem constants (hardcoded; must match the reference) ----
B = 64
D = 128
S_DIM = 1024
T_DIM = 2048
NCE_K = 16384
KP1 = NCE_K + 1          # 16385
N_DATA = 100000
NCE_T = 0.07
EPS = 1e-7
PN = 1.0 / N_DATA
CVAL = NCE_K * PN + EPS  # c = m*Pn + eps

N_CORES = 8
W = 512                  # matmul window along n (psum-bank aligned)
N_WIN = 25
R = N_WIN * W            # 12800 padded bank rows per core (12500 real)
N_PAD = N_CORES * R      # 102400 padded table rows
GRPS = [4, 4, 4, 4, 4, 4, 1]   # windows per group (4 = one PSUM quad)
# per-group flavor: 'L' = ln(cnt) folded into the exponent via an identity
# matmul (PE) with M1 from the activation accumulator; 'C' = cnt shipped
# instead of L, weighting on the vector engine (spreads load PE<->DVE)
FLAVORS = "LLLLLLL"
M2_STRIDE = 16
BANK_SCALE = 32.0        # bank values pre-scaled for fp8e4m3
WSCALE = 16.0            # projection weights pre-scaled for fp8e4m3
LCLAMP = 240.0           # fp8e4m3 max; cnt=0 slots get L = -240

F32 = mybir.dt.float32
BF16 = mybir.dt.bfloat16
FP8 = mybir.dt.float8e4
U32 = mybir.dt.uint32

TRACE = False            # test.py can flip this for profiling runs
_CACHE = {}


def _build_program():
    nc = bacc.Bacc("TRN2", target_bir_lowering=False, debug=False,
                   num_devices=N_CORES)

    # ---- I/O ----
    # emb8 pack (fp8): [0:1024 wt_s | 1024:1536 ft_s | 1536:3584 wt_t |
    #                   3584:4608 ft_t | 4608:4736 identity]
    emb8 = nc.dram_tensor("emb8", [D, 4736], FP8, kind="ExternalInput")
    # f32 pack: [0 bs | 1 bt | 2:66 p1 | 66:130 p2 | 130:194 rb_s |
    #            194:258 rb_t | 258 escale]
    fpk = nc.dram_tensor("fpk", [D, 259], F32, kind="ExternalInput")
    # fused bank+L stream: per group, the interleaved bank windows
    # [(w k j), k=0 -> 32*mem2T, k=1 -> 32*mem1T] followed by the L cols
    memfl = nc.dram_tensor("memfl", [D, 3 * R], FP8, kind="ExternalInput")
    out_acc = nc.dram_tensor("out_acc", [D, 8], F32, kind="ExternalOutput")

    gpos = [0]
    for x in GRPS:
        gpos.append(gpos[-1] + x)

    with tile.TileContext(nc) as tc:
        with tc.tile_pool(name="persist", bufs=1) as pp, \
             tc.tile_pool(name="grp", bufs=3) as gp, \
             tc.tile_pool(name="psum", bufs=2, space="PSUM") as psp:

            # ---- ALL input DMAs on the sync engine's HWDGE ring, in
            # priority order: ring-FIFO means the embed pack and the first
            # bank/L chunks get the full HBM bandwidth before later chunks,
            # and the 0.6us-per-dma_start trigger cost lands on the
            # otherwise idle sync engine, not on ACT ----
            emb8_t = pp.tile([D, 4736], FP8, tag="emb8")
            nc.sync.dma_start(out=emb8_t[:, 0:1536], in_=emb8[:, 0:1536])
            fpk_t = pp.tile([D, 259], F32, tag="fpk")
            nc.sync.dma_start(out=fpk_t[:], in_=fpk[:])

            # one fused tile holds both streams; per group one DMA.
            # group g occupies cols [gpos[g]*1536, gpos[g+1]*1536): first
            # ng*1024 bank bytes (w-major, k-interleaved), then ng*512 L
            fl_t = pp.tile([D, 3 * R], FP8, tag="fl")
            for g in range(len(GRPS)):
                csl = slice(gpos[g] * 3 * W, gpos[g + 1] * 3 * W)
                eng = nc.sync
                eng.dma_start(out=fl_t[:, csl], in_=memfl[:, csl])
                if g == 0:
                    # the t-side embed inputs ride between chunk 0 and 1
                    nc.sync.dma_start(out=emb8_t[:, 1536:4736],
                                      in_=emb8[:, 1536:4736])

            def bank_ap(w):
                g = next(i for i in range(len(GRPS))
                         if gpos[i] <= w < gpos[i + 1])
                base = gpos[g] * 3 * W + (w - gpos[g]) * 2 * W
                return fl_t[:, base:base + 2 * W].rearrange(
                    "p (k j) -> p k j", k=2)

            def l_ap(w):
                g = next(i for i in range(len(GRPS))
                         if gpos[i] <= w < gpos[i + 1])
                ng = GRPS[g]
                base = (gpos[g] * 3 + ng * 2) * W + (w - gpos[g]) * W
                return fl_t[:, base:base + W]

            wt_s = emb8_t[:, 0:1024].rearrange("p (c d) -> p c d", c=8)
            ft_s = emb8_t[:, 1024:1536].rearrange("p (c b) -> p c b", c=8)
            wt_t = emb8_t[:, 1536:3584].rearrange("p (c d) -> p c d", c=16)
            ft_t = emb8_t[:, 3584:4608].rearrange("p (c b) -> p c b", c=16)
            id_t = emb8_t[:, 4608:4736]
            bs_t = fpk_t[:, 0:1]
            bt_t = fpk_t[:, 1:2]
            p1 = fpk_t[:, 2:66]
            p2 = fpk_t[:, 66:130]
            rb_s = fpk_t[:, 130:194]
            rb_t = fpk_t[:, 194:258]
            escale = fpk_t[:, 258:259]

            # ---- constants ----
            # trigger the Exp table load early, overlapped with the DMAs
            dummy = pp.tile([1, 1], BF16, tag="dummy")
            nc.vector.memset(dummy[:], 1.0)
            nc.scalar.activation(out=dummy[:], in_=dummy[:],
                                 func=mybir.ActivationFunctionType.Exp)

            # ---- PE warm-up (part 1) ----
            wz_l = pp.tile([D, D], BF16, tag="wz_l")
            wz_r = pp.tile([D, W], BF16, tag="wz_r")
            nc.vector.memset(wz_l[:], 0.0)
            nc.vector.memset(wz_r[:], 0.0)
            wz_p = psp.tile([D, W], F32, tag="q", name="wz_p",
                            padded_shape=[D, 4 * W])
            for _wu in range(3):
                nc.tensor.matmul(out=wz_p[:], lhsT=wz_l[:], rhs=wz_r[:],
                                 start=True, stop=True)

            # ---- embed: vraw.T = (W f).T + b -> [D, B]; 1/||v|| and its
            # [D, B] broadcast come in precomputed (fpk pack) ----
            def embed(wt, ft, bt_, rb, n_chunks, tag):
                vps = psp.tile([D, B], F32, tag="q", name=f"vps_{tag}",
                               padded_shape=[D, 4 * W])
                for c in range(n_chunks):
                    nc.tensor.matmul(out=vps[:], lhsT=wt[:, c, :],
                                     rhs=ft[:, c, :],
                                     start=(c == 0), stop=(c == n_chunks - 1))
                vraw = pp.tile([D, B], F32, tag=f"vraw_{tag}")
                nc.vector.tensor_scalar(out=vraw[:], in0=vps[:],
                                        scalar1=float(1.0 / WSCALE),
                                        scalar2=bt_,
                                        op0=mybir.AluOpType.mult,
                                        op1=mybir.AluOpType.add)
                vT = pp.tile([D, B], F32, tag=f"vT_{tag}")
                nc.vector.tensor_tensor(out=vT[:], in0=vraw[:], in1=rb,
                                        op=mybir.AluOpType.mult)
                return vT, vraw

            vTs, vraw_s = embed(wt_s, ft_s, bs_t, rb_s, S_DIM // D, "s")
            vTt, vraw_t = embed(wt_t, ft_t, bt_t, rb_t, T_DIM // D, "t")



            # fused DoubleRow weights [128, (k m)] fp8:
            # k=0 slot: cols 0:64 = vraw_s, else 0; k=1: cols 64:128 = vraw_t
            wfused = pp.tile([D, 2 * D], FP8, tag="wfused")
            nc.vector.memset(wfused[:], 0.0)
            nc.vector.tensor_copy(out=wfused[:, 0:B], in_=vraw_s[:])
            nc.vector.tensor_copy(out=wfused[:, D + B:2 * D], in_=vraw_t[:])
            wf3 = wfused[:].rearrange("p (k m) -> p k m", k=2)

            # ---- PE warm-up (part 2): depends on wfused, so it fills the
            # PE gap between the embed and the first bank chunk's arrival,
            # keeping the HAM clock ramped ----
            wz_r8 = pp.tile([D, W], FP8, tag="wz_r8")
            nc.vector.memset(wz_r8[:], 0.0)
            for _wu in range(5):
                nc.tensor.matmul(out=wz_p[:], lhsT=wfused[:, 0:D],
                                 rhs=wz_r8[:], start=True, stop=True)

            # ---- output tile doubles as the moment accumulators:
            # col 0 = M1, col 1 = M2L', col 2 = M2C, col 3/4 = pacc ----
            ot = pp.tile([D, 8], F32)
            nc.vector.memset(ot[:], 0.0)
            macc = [ot[:, m:m + 1] for m in range(3)]

            # ---- positives: pacc into ot cols 3/4 ----
            pscr = pp.tile([D, B], F32, tag="pscr")
            pscr2 = pp.tile([D, B], F32, tag="pscr2")

            nc.vector.scalar_tensor_tensor(
                out=pscr[:], in0=p2, scalar=1.0, in1=vTs[:],
                op0=mybir.AluOpType.mult, op1=mybir.AluOpType.mult,
                accum_out=ot[:, 3:4])
            nc.vector.scalar_tensor_tensor(
                out=pscr2[:], in0=p1, scalar=1.0, in1=vTt[:],
                op0=mybir.AluOpType.mult, op1=mybir.AluOpType.mult,
                accum_out=ot[:, 4:5])

            # ---- main loop: per window one DoubleRow matmul (scores for
            # both sides) + one identity matmul adding L into the same
            # bank; per quad one exp with fused M1 accumulation ----
            for g, ng in enumerate(GRPS):
                GWg = ng * W
                flavor = FLAVORS[g]
                quad = psp.tile([D, GWg], F32, tag="q", name=f"quad_{g}",
                                padded_shape=[D, 4 * W])
                for j in range(ng):
                    w = gpos[g] + j
                    jsl = slice(j * W, (j + 1) * W)
                    nc.tensor.matmul(out=quad[:, jsl],
                                     lhsT=wf3, rhs=bank_ap(w),
                                     start=True, stop=(flavor == "C"),
                                     perf_mode=mybir.MatmulPerfMode.DoubleRow)
                    if flavor == "L":
                        nc.tensor.matmul(out=quad[:, jsl], lhsT=id_t[:],
                                         rhs=l_ap(w),
                                         start=False, stop=True)
                eg = gp.tile([D, GWg], BF16, tag="e_g", name=f"eg_{g}",
                             padded_shape=[D, 4 * W])
                acc1 = gp.tile([D, 1], F32, tag="acc1", name=f"acc1_{g}")
                acc2 = gp.tile([D, 1], F32, tag="acc2", name=f"acc2_{g}")
                u2 = gp.tile([D, GWg // M2_STRIDE], BF16, tag="u2",
                             name=f"u2_{g}",
                             padded_shape=[D, 4 * W // M2_STRIDE])
                if flavor == "L":
                    # exp gives cnt*e; M1 from the ACT accumulator,
                    # M2' = sum (cnt*e)^2 on a stride-16 subsample
                    nc.scalar.activation(out=eg[:], in_=quad[:],
                                         func=mybir.ActivationFunctionType.Exp,
                                         scale=escale,
                                         accum_out=acc1[:])
                    nc.vector.scalar_tensor_tensor(
                        out=u2[:], in0=eg[:, 0:GWg:M2_STRIDE], scalar=1.0,
                        in1=eg[:, 0:GWg:M2_STRIDE],
                        op0=mybir.AluOpType.mult, op1=mybir.AluOpType.mult,
                        accum_out=acc2[:])
                    m2dst = macc[1]
                else:
                    # exp gives bare e; cnt weighting on the DVE:
                    # u1 = cnt*e (accum -> M1), u2 = u1*e (accum -> M2 exact)
                    nc.scalar.activation(out=eg[:], in_=quad[:],
                                         func=mybir.ActivationFunctionType.Exp,
                                         scale=escale[:, 0:1])
                    u1 = gp.tile([D, GWg], BF16, tag="u1", name=f"u1_{g}",
                                 padded_shape=[D, 4 * W])
                    nc.vector.scalar_tensor_tensor(
                        out=u1[:], in0=eg[:], scalar=1.0, in1=l_ap(gpos[g]),
                        op0=mybir.AluOpType.mult, op1=mybir.AluOpType.mult,
                        accum_out=acc1[:])
                    nc.vector.scalar_tensor_tensor(
                        out=u2[:], in0=u1[:, 0:GWg:M2_STRIDE], scalar=1.0,
                        in1=eg[:, 0:GWg:M2_STRIDE],
                        op0=mybir.AluOpType.mult, op1=mybir.AluOpType.mult,
                        accum_out=acc2[:])
                    m2dst = macc[2]
                nc.vector.tensor_tensor(out=macc[0], in0=macc[0],
                                        in1=acc1[:], op=mybir.AluOpType.add)
                nc.vector.tensor_tensor(out=m2dst, in0=m2dst,
                                        in1=acc2[:], op=mybir.AluOpType.add)

            # ---- output: pacc into ot cols 3/4, then one small DMA ----
            nc.scalar.dma_start(out=out_acc[:], in_=ot[:])

    nc.finalize()
    return nc


def _prepare_in_maps(f_s, f_t, idx, contrast_idx, Ws, bs, Wt, bt,
                     memory_v1, memory_v2):
    f_s = np.asarray(f_s, dtype=np.float32)
    f_t = np.asarray(f_t, dtype=np.float32)
    Ws = np.asarray(Ws, dtype=np.float32)
    Wt = np.asarray(Wt, dtype=np.float32)
    bs = np.asarray(bs, dtype=np.float32)
    bt = np.asarray(bt, dtype=np.float32)
    memory_v1 = np.asarray(memory_v1, dtype=np.float32)
    memory_v2 = np.asarray(memory_v2, dtype=np.float32)
    idx = np.asarray(idx).astype(np.int64)
    contrast_idx = np.asarray(contrast_idx).astype(np.int64)

    fp8 = ml_dtypes.float8_e4m3fn

    # ---- index prep (sharding metadata): multiplicity counts ----
    idx_all = np.concatenate([idx[:, None], contrast_idx[:, 1:]], axis=1)
    counts = np.zeros((B, N_DATA), dtype=np.float32)
    brow = np.repeat(np.arange(B), KP1)
    np.add.at(counts, (brow, idx_all.ravel()), 1.0)
    # mask of n-columns handled by 'L'-flavor groups (same window layout
    # in every core's shard)
    gpos = [0]
    for x in GRPS:
        gpos.append(gpos[-1] + x)
    gpos_pairs = [(gpos[i], gpos[i + 1]) for i in range(len(GRPS))]
    lwin = np.zeros(R, dtype=bool)
    for g, f in enumerate(FLAVORS):
        if f == "L":
            lwin[gpos[g] * W:gpos[g + 1] * W] = True
    lmask_n = np.zeros(N_PAD, dtype=bool)
    for c in range(N_CORES):
        lmask_n[c * R:(c + 1) * R] = lwin
    cl = counts[:, lmask_n[:N_DATA]].astype(np.float64)
    _CACHE["m2_ratio"] = float(cl.sum()) / max(float((cl ** 2).sum()), 1.0)

    # ---- replicated small tensors (weights/features scaled for fp8) ----
    def arrange(mT, cols, scale):
        n_chunks = mT.shape[0] // D
        a = mT.reshape(n_chunks, D, cols).transpose(1, 0, 2).reshape(D, -1)
        return np.ascontiguousarray((a * scale).astype(fp8))

    wsT = arrange(Ws.T, D, WSCALE)
    wtT = arrange(Wt.T, D, WSCALE)
    fsT = arrange(f_s.T, B, 1.0)
    ftT = arrange(f_t.T, B, 1.0)
    pos1T = np.ascontiguousarray(memory_v1[idx].T)
    pos2T = np.ascontiguousarray(memory_v2[idx].T)
    ident = np.eye(D, dtype=np.float32).astype(fp8)
    emb8 = np.concatenate([wsT, fsT, wtT, ftT, ident], axis=1)
    assert emb8.shape == (D, 4736), emb8.shape

    # host mirror of the device embed (same fp8-quantized operands the PE
    # sees, so rinv matches the device vraw to fp32-accumulation noise)
    def host_rinv(wq, fq, b):
        vraw = (fq.astype(np.float32) @ wq.astype(np.float32).T) / WSCALE + b
        return (1.0 / np.sqrt((vraw ** 2).sum(axis=1))).astype(np.float32)

    rinv_s = host_rinv((Ws * WSCALE).astype(fp8), f_s.astype(fp8), bs)
    rinv_t = host_rinv((Wt * WSCALE).astype(fp8), f_t.astype(fp8), bt)
    esc_s = rinv_s.astype(np.float64) / (NCE_T * BANK_SCALE)
    esc_t = rinv_t.astype(np.float64) / (NCE_T * BANK_SCALE)
    escv = np.concatenate([esc_s, esc_t]).astype(np.float32).reshape(D, 1)
    fpk = np.zeros((D, 259), dtype=np.float32)
    fpk[:, 0] = bs
    fpk[:, 1] = bt
    fpk[:, 2:66] = pos1T
    fpk[:, 66:130] = pos2T
    fpk[:, 130:194] = np.tile(rinv_s.reshape(1, B), (D, 1))
    fpk[:, 194:258] = np.tile(rinv_t.reshape(1, B), (D, 1))
    fpk[:, 258:259] = escv

    # L-windows: L[row, n] = ln(cnt)/escale_row (cnt=0 -> -LCLAMP);
    # C-windows: the raw count (exact small ints in fp8)
    with np.errstate(divide="ignore"):
        lncnt = np.log(counts)          # [B, N_DATA], -inf where cnt=0
    L_s = lncnt / esc_s[:, None]
    L_t = lncnt / esc_t[:, None]
    L = np.concatenate([L_s, L_t], axis=0)      # [128, N_DATA]
    L = np.clip(L, -LCLAMP, LCLAMP)
    L8 = np.full((D, N_PAD), -LCLAMP, dtype=np.float32)
    L8[:, :N_DATA] = L
    cnt2 = np.zeros((D, N_PAD), dtype=np.float32)
    cnt2[:B, :N_DATA] = counts
    cnt2[B:, :N_DATA] = counts
    L8[:, ~lmask_n] = cnt2[:, ~lmask_n]
    L8 = L8.astype(fp8)

    # pad the n dimension to N_PAD (zero rows: L=-240 kills them)
    def pad_cols(a):
        out = np.zeros((a.shape[0], N_PAD), dtype=fp8)
        out[:, :N_DATA] = a
        return out

    m1s = pad_cols((memory_v1.T * BANK_SCALE).astype(fp8))
    m2s = pad_cols((memory_v2.T * BANK_SCALE).astype(fp8))

    in_maps = []
    for c in range(N_CORES):
        sl = slice(c * R, (c + 1) * R)
        # fused interleaved banks [D, N_WIN, 2, W]: k=0 -> mem2 (s side),
        # k=1 -> mem1 (t side)
        mf = np.empty((D, N_WIN, 2, W), dtype=fp8)
        mf[:, :, 0, :] = m2s[:, sl].reshape(D, N_WIN, W)
        mf[:, :, 1, :] = m1s[:, sl].reshape(D, N_WIN, W)
        Lc = L8[:, sl]
        # fused group-contiguous stream: per group, bank bytes then L bytes
        memfl = np.empty((D, 3 * R), dtype=fp8)
        for g, ng in enumerate(gpos_pairs):
            g0, g1 = ng
            base = g0 * 3 * W
            nb = (g1 - g0) * 2 * W
            memfl[:, base:base + nb] = mf[:, g0:g1].reshape(D, nb)
            memfl[:, base + nb:base + nb + (g1 - g0) * W] = \
                Lc[:, g0 * W:g1 * W]
        in_maps.append({
            "emb8": emb8, "fpk": fpk,
            "memfl": np.ascontiguousarray(memfl),
        })
    return in_maps


def _combine(out_accs):
    """out_accs: per-core [128, 8] float arrays -> scalar loss (float32)."""
    outs = [np.asarray(o).astype(np.float64) for o in out_accs]
    m2_ratio = _CACHE["m2_ratio"]

    def side_loss(half, possum):
        # M1 = sum cnt*e ; M2 = stride * (ratio*sum(cnt*e)^2 + sum cnt*e^2)
        M = [sum(o[half, m].sum() for o in outs) for m in range(3)]
        M[1] = float(M2_STRIDE) * (m2_ratio * M[1] + M[2])
        Z = M[0] / (B * KP1) * N_DATA
        cz = CVAL * Z
        # sum cnt*ln(x+c) = B*KP1*ln(c) + sum_m (-1)^(m+1) M_m/(m cz^m)
        series = sum((-1.0) ** m * M[m] / ((m + 1) * cz ** (m + 1))
                     for m in range(2))
        sum_ln_xc = B * KP1 * np.log(CVAL) + series
        neg_b_loss = (possum / NCE_T - B * np.log(Z)
                      + B * NCE_K * np.log(NCE_K * PN) - sum_ln_xc)
        return -neg_b_loss / B

    s_loss = side_loss(slice(0, B), outs[0][:, 3].sum())
    t_loss = side_loss(slice(B, D), outs[0][:, 4].sum())
    return np.float32(s_loss + t_loss)


def kernel(f_s, f_t, idx, contrast_idx, Ws, bs, Wt, bt, memory_v1, memory_v2):
    in_maps = _prepare_in_maps(f_s, f_t, idx, contrast_idx, Ws, bs, Wt, bt,
                               memory_v1, memory_v2)
    if "nc" not in _CACHE:
        _CACHE["nc"] = _build_program()
    nc = _CACHE["nc"]
    res = run_bass_kernel_spmd(nc, in_maps, list(range(N_CORES)), trace=TRACE)
    _CACHE["last_results"] = res
    return kernel_combine_results(res)


def kernel_combine_results(res):
    return _combine([res.results[c]["out_acc"] for c in range(N_CORES)])


# revision 16
# speedup vs baseline: 1.0843x; 1.0843x over previous
"""CRCDLoss Trainium2 kernel (8-core SPMD, Bass/Tile), v3.

Core strategy: replace the reference's per-(b,k) gather (~1.07 GB) with a
dense score matmul S[b, n] = v_b . mem_n over the full memory banks,
sharded along n across the 8 cores. The multiplicity counts
cnt[b, n] = #{k : idx_all[b,k] == n} are folded INTO THE EXPONENT:
one DoubleRow fp8 matmul per 512-col window computes both sides' scores
(PSUM rows 0:64 = v_s.mem2, 64:128 = v_t.mem1) and a second plain
identity matmul accumulates L[b, n] = ln(cnt)/escale_b into the same
PSUM bank, so that the activation exp(escale*(S+L)) = cnt * e directly.
The activation's free accum_out then yields the moment M1 = sum cnt*e
with no vector-engine pass at all. M2 is estimated from a stride-16
subsample of (cnt*e)^2 rescaled by the host ratio sum(cnt)/sum(cnt^2)
(valid since cnt and e are independent), entering only a tiny 2nd-order
series term. The Z normalizer is eliminated algebraically via the
2-term log series; the host combines per-core sums in float64.
Positives use host-gathered f32 rows against the normalized v.

Per-core HBM traffic: banks fused fp8 3.28MB + L fp8 1.64MB + embed
fp8 0.6MB => ~5.6MB (~16us at 358 GB/s), vs 9.8MB bf16 in v1.
"""

import sys

import numpy as np

try:
    import concourse.bass as bass  # noqa: F401
except ImportError:
    sys.path.insert(0, "/opt/trn_rl_repo")

import concourse.bacc as bacc
import concourse.bass as bass  # noqa: F811
import concourse.mybir as mybir
import concourse.tile as tile
from concourse.bass_utils import run_bass_kernel_spmd

import ml_dtypes

# ---- problem constants (hardcoded; must match the reference) ----
B = 64
D = 128
S_DIM = 1024
T_DIM = 2048
NCE_K = 16384
KP1 = NCE_K + 1          # 16385
N_DATA = 100000
NCE_T = 0.07
EPS = 1e-7
PN = 1.0 / N_DATA
CVAL = NCE_K * PN + EPS  # c = m*Pn + eps

N_CORES = 8
W = 512                  # matmul window along n (psum-bank aligned)
N_WIN = 25
R = N_WIN * W            # 12800 padded bank rows per core (12500 real)
N_PAD = N_CORES * R      # 102400 padded table rows
GRPS = [4, 4, 4, 4, 4, 4, 1]   # windows per group (4 = one PSUM quad)
# per-group flavor: 'L' = ln(cnt) folded into the exponent via an identity
# matmul (PE) with M1 from the activation accumulator; 'C' = cnt shipped
# instead of L, weighting on the vector engine (spreads load PE<->DVE)
FLAVORS = "LLLLLLL"
M2_STRIDE = 16
BANK_SCALE = 32.0        # bank values pre-scaled for fp8e4m3
WSCALE = 16.0            # projection weights pre-scaled for fp8e4m3
LCLAMP = 240.0           # fp8e4m3 max; cnt=0 slots get L = -240

F32 = mybir.dt.float32
BF16 = mybir.dt.bfloat16
FP8 = mybir.dt.float8e4
U32 = mybir.dt.uint32

TRACE = False            # test.py can flip this for profiling runs
_CACHE = {}


def _build_program():
    nc = bacc.Bacc("TRN2", target_bir_lowering=False, debug=False,
                   num_devices=N_CORES)

    # ---- I/O ----
    # emb8 pack (fp8): [0:1024 wt_s | 1024:1536 ft_s | 1536:3584 wt_t |
    #                   3584:4608 ft_t | 4608:4736 identity]
    emb8 = nc.dram_tensor("emb8", [D, 4736], FP8, kind="ExternalInput")
    # f32 pack: [0 bs | 1 bt | 2:66 p1 | 66:130 p2 | 130:194 rb_s |
    #            194:258 rb_t | 258 escale]
    fpk = nc.dram_tensor("fpk", [D, 259], F32, kind="ExternalInput")
    # fused interleaved banks: [d, (w k j)] with k=0 -> 32*mem2T (s side),
    # k=1 -> 32*mem1T (t side)
    memf = nc.dram_tensor("memf", [D, 2 * R], FP8, kind="ExternalInput")
    lnc = nc.dram_tensor("lnc", [D, R], FP8, kind="ExternalInput")
    out_acc = nc.dram_tensor("out_acc", [D, 8], F32, kind="ExternalOutput")

    gpos = [0]
    for x in GRPS:
        gpos.append(gpos[-1] + x)

    with tile.TileContext(nc) as tc:
        with tc.tile_pool(name="persist", bufs=1) as pp, \
             tc.tile_pool(name="grp", bufs=3) as gp, \
             tc.tile_pool(name="psum", bufs=2, space="PSUM") as psp:

            # ---- ALL input DMAs on the sync engine's HWDGE ring, in
            # priority order: ring-FIFO means the embed pack and the first
            # bank/L chunks get the full HBM bandwidth before later chunks,
            # and the 0.6us-per-dma_start trigger cost lands on the
            # otherwise idle sync engine, not on ACT ----
            emb8_t = pp.tile([D, 4736], FP8, tag="emb8")
            nc.sync.dma_start(out=emb8_t[:], in_=emb8[:])
            fpk_t = pp.tile([D, 259], F32, tag="fpk")
            nc.sync.dma_start(out=fpk_t[:], in_=fpk[:])

            bank_t = pp.tile([D, N_WIN, 2, W], FP8, tag="bank")
            lnc_t = pp.tile([D, R], FP8, tag="lnc")
            memf_v = memf[:].rearrange("p (w k j) -> p w k j", w=N_WIN, k=2)
            for g in range(len(GRPS)):
                gsl = slice(gpos[g], gpos[g + 1])
                csl = slice(gpos[g] * W, gpos[g + 1] * W)
                nc.sync.dma_start(out=bank_t[:, gsl], in_=memf_v[:, gsl])
                nc.sync.dma_start(out=lnc_t[:, csl], in_=lnc[:, csl])

            def bank_ap(w):
                return bank_t[:, w]

            def l_ap(w):
                return lnc_t[:, w * W:(w + 1) * W]

            wt_s = emb8_t[:, 0:1024].rearrange("p (c d) -> p c d", c=8)
            ft_s = emb8_t[:, 1024:1536].rearrange("p (c b) -> p c b", c=8)
            wt_t = emb8_t[:, 1536:3584].rearrange("p (c d) -> p c d", c=16)
            ft_t = emb8_t[:, 3584:4608].rearrange("p (c b) -> p c b", c=16)
            id_t = emb8_t[:, 4608:4736]
            bs_t = fpk_t[:, 0:1]
            bt_t = fpk_t[:, 1:2]
            p1 = fpk_t[:, 2:66]
            p2 = fpk_t[:, 66:130]
            rb_s = fpk_t[:, 130:194]
            rb_t = fpk_t[:, 194:258]
            escale = fpk_t[:, 258:259]

            # ---- constants ----
            # trigger the Exp table load early, overlapped with the DMAs
            dummy = pp.tile([1, 1], BF16, tag="dummy")
            nc.vector.memset(dummy[:], 1.0)
            nc.scalar.activation(out=dummy[:], in_=dummy[:],
                                 func=mybir.ActivationFunctionType.Exp)

            # ---- PE warm-up (part 1) ----
            wz_l = pp.tile([D, D], BF16, tag="wz_l")
            wz_r = pp.tile([D, W], BF16, tag="wz_r")
            nc.vector.memset(wz_l[:], 0.0)
            nc.vector.memset(wz_r[:], 0.0)
            wz_p = psp.tile([D, W], F32, tag="q", name="wz_p",
                            padded_shape=[D, 4 * W])
            for _wu in range(3):
                nc.tensor.matmul(out=wz_p[:], lhsT=wz_l[:], rhs=wz_r[:],
                                 start=True, stop=True)

            # ---- embed: vraw.T = (W f).T + b -> [D, B]; 1/||v|| and its
            # [D, B] broadcast come in precomputed (fpk pack) ----
            def embed(wt, ft, bt_, rb, n_chunks, tag):
                vps = psp.tile([D, B], F32, tag="q", name=f"vps_{tag}",
                               padded_shape=[D, 4 * W])
                for c in range(n_chunks):
                    nc.tensor.matmul(out=vps[:], lhsT=wt[:, c, :],
                                     rhs=ft[:, c, :],
                                     start=(c == 0), stop=(c == n_chunks - 1))
                vraw = pp.tile([D, B], F32, tag=f"vraw_{tag}")
                nc.vector.tensor_scalar(out=vraw[:], in0=vps[:],
                                        scalar1=float(1.0 / WSCALE),
                                        scalar2=bt_,
                                        op0=mybir.AluOpType.mult,
                                        op1=mybir.AluOpType.add)
                vT = pp.tile([D, B], F32, tag=f"vT_{tag}")
                nc.vector.tensor_tensor(out=vT[:], in0=vraw[:], in1=rb,
                                        op=mybir.AluOpType.mult)
                return vT, vraw

            vTs, vraw_s = embed(wt_s, ft_s, bs_t, rb_s, S_DIM // D, "s")
            vTt, vraw_t = embed(wt_t, ft_t, bt_t, rb_t, T_DIM // D, "t")



            # fused DoubleRow weights [128, (k m)] fp8:
            # k=0 slot: cols 0:64 = vraw_s, else 0; k=1: cols 64:128 = vraw_t
            wfused = pp.tile([D, 2 * D], FP8, tag="wfused")
            nc.vector.memset(wfused[:], 0.0)
            nc.vector.tensor_copy(out=wfused[:, 0:B], in_=vraw_s[:])
            nc.vector.tensor_copy(out=wfused[:, D + B:2 * D], in_=vraw_t[:])
            wf3 = wfused[:].rearrange("p (k m) -> p k m", k=2)

            # ---- PE warm-up (part 2): depends on wfused, so it fills the
            # PE gap between the embed and the first bank chunk's arrival,
            # keeping the HAM clock ramped ----
            wz_r8 = pp.tile([D, W], FP8, tag="wz_r8")
            nc.vector.memset(wz_r8[:], 0.0)
            for _wu in range(5):
                nc.tensor.matmul(out=wz_p[:], lhsT=wfused[:, 0:D],
                                 rhs=wz_r8[:], start=True, stop=True)

            # ---- output tile doubles as the moment accumulators:
            # col 0 = M1, col 1 = M2L', col 2 = M2C, col 3/4 = pacc ----
            ot = pp.tile([D, 8], F32)
            nc.vector.memset(ot[:], 0.0)
            macc = [ot[:, m:m + 1] for m in range(3)]

            # ---- positives: pacc into ot cols 3/4 ----
            pscr = pp.tile([D, B], F32, tag="pscr")
            pscr2 = pp.tile([D, B], F32, tag="pscr2")

            nc.vector.scalar_tensor_tensor(
                out=pscr[:], in0=p2, scalar=1.0, in1=vTs[:],
                op0=mybir.AluOpType.mult, op1=mybir.AluOpType.mult,
                accum_out=ot[:, 3:4])
            nc.vector.scalar_tensor_tensor(
                out=pscr2[:], in0=p1, scalar=1.0, in1=vTt[:],
                op0=mybir.AluOpType.mult, op1=mybir.AluOpType.mult,
                accum_out=ot[:, 4:5])

            # ---- main loop: per window one DoubleRow matmul (scores for
            # both sides) + one identity matmul adding L into the same
            # bank; per quad one exp with fused M1 accumulation ----
            for g, ng in enumerate(GRPS):
                GWg = ng * W
                flavor = FLAVORS[g]
                quad = psp.tile([D, GWg], F32, tag="q", name=f"quad_{g}",
                                padded_shape=[D, 4 * W])
                for j in range(ng):
                    w = gpos[g] + j
                    jsl = slice(j * W, (j + 1) * W)
                    nc.tensor.matmul(out=quad[:, jsl],
                                     lhsT=wf3, rhs=bank_ap(w),
                                     start=True, stop=(flavor == "C"),
                                     perf_mode=mybir.MatmulPerfMode.DoubleRow)
                    if flavor == "L":
                        nc.tensor.matmul(out=quad[:, jsl], lhsT=id_t[:],
                                         rhs=l_ap(w),
                                         start=False, stop=True)
                eg = gp.tile([D, GWg], BF16, tag="e_g", name=f"eg_{g}",
                             padded_shape=[D, 4 * W])
                acc1 = gp.tile([D, 1], F32, tag="acc1", name=f"acc1_{g}")
                acc2 = gp.tile([D, 1], F32, tag="acc2", name=f"acc2_{g}")
                u2 = gp.tile([D, GWg // M2_STRIDE], BF16, tag="u2",
                             name=f"u2_{g}",
                             padded_shape=[D, 4 * W // M2_STRIDE])
                if flavor == "L":
                    # exp gives cnt*e; M1 from the ACT accumulator,
                    # M2' = sum (cnt*e)^2 on a stride-16 subsample
                    nc.scalar.activation(out=eg[:], in_=quad[:],
                                         func=mybir.ActivationFunctionType.Exp,
                                         scale=escale,
                                         accum_out=acc1[:])
                    nc.vector.scalar_tensor_tensor(
                        out=u2[:], in0=eg[:, 0:GWg:M2_STRIDE], scalar=1.0,
                        in1=eg[:, 0:GWg:M2_STRIDE],
                        op0=mybir.AluOpType.mult, op1=mybir.AluOpType.mult,
                        accum_out=acc2[:])
                    m2dst = macc[1]
                else:
                    # exp gives bare e; cnt weighting on the DVE:
                    # u1 = cnt*e (accum -> M1), u2 = u1*e (accum -> M2 exact)
                    nc.scalar.activation(out=eg[:], in_=quad[:],
                                         func=mybir.ActivationFunctionType.Exp,
                                         scale=escale[:, 0:1])
                    u1 = gp.tile([D, GWg], BF16, tag="u1", name=f"u1_{g}",
                                 padded_shape=[D, 4 * W])
                    nc.vector.scalar_tensor_tensor(
                        out=u1[:], in0=eg[:], scalar=1.0, in1=l_ap(gpos[g]),
                        op0=mybir.AluOpType.mult, op1=mybir.AluOpType.mult,
                        accum_out=acc1[:])
                    nc.vector.scalar_tensor_tensor(
                        out=u2[:], in0=u1[:, 0:GWg:M2_STRIDE], scalar=1.0,
                        in1=eg[:, 0:GWg:M2_STRIDE],
                        op0=mybir.AluOpType.mult, op1=mybir.AluOpType.mult,
                        accum_out=acc2[:])
                    m2dst = macc[2]
                nc.vector.tensor_tensor(out=macc[0], in0=macc[0],
                                        in1=acc1[:], op=mybir.AluOpType.add)
                nc.vector.tensor_tensor(out=m2dst, in0=m2dst,
                                        in1=acc2[:], op=mybir.AluOpType.add)

            # ---- output: the pacc columns are ready long before the
            # moments; ship them early so the final DMA is minimal ----
            nc.scalar.dma_start(out=out_acc[:, 3:8], in_=ot[:, 3:8])
            nc.scalar.dma_start(out=out_acc[:, 0:3], in_=ot[:, 0:3])

    nc.finalize()
    return nc


def _prepare_in_maps(f_s, f_t, idx, contrast_idx, Ws, bs, Wt, bt,
                     memory_v1, memory_v2):
    f_s = np.asarray(f_s, dtype=np.float32)
    f_t = np.asarray(f_t, dtype=np.float32)
    Ws = np.asarray(Ws, dtype=np.float32)
    Wt = np.asarray(Wt, dtype=np.float32)
    bs = np.asarray(bs, dtype=np.float32)
    bt = np.asarray(bt, dtype=np.float32)
    memory_v1 = np.asarray(memory_v1, dtype=np.float32)
    memory_v2 = np.asarray(memory_v2, dtype=np.float32)
    idx = np.asarray(idx).astype(np.int64)
    contrast_idx = np.asarray(contrast_idx).astype(np.int64)

    fp8 = ml_dtypes.float8_e4m3fn

    # ---- index prep (sharding metadata): multiplicity counts ----
    idx_all = np.concatenate([idx[:, None], contrast_idx[:, 1:]], axis=1)
    counts = np.zeros((B, N_DATA), dtype=np.float32)
    brow = np.repeat(np.arange(B), KP1)
    np.add.at(counts, (brow, idx_all.ravel()), 1.0)
    # mask of n-columns handled by 'L'-flavor groups (same window layout
    # in every core's shard)
    gpos = [0]
    for x in GRPS:
        gpos.append(gpos[-1] + x)
    gpos_pairs = [(gpos[i], gpos[i + 1]) for i in range(len(GRPS))]
    lwin = np.zeros(R, dtype=bool)
    for g, f in enumerate(FLAVORS):
        if f == "L":
            lwin[gpos[g] * W:gpos[g + 1] * W] = True
    lmask_n = np.zeros(N_PAD, dtype=bool)
    for c in range(N_CORES):
        lmask_n[c * R:(c + 1) * R] = lwin
    cl = counts[:, lmask_n[:N_DATA]].astype(np.float64)
    _CACHE["m2_ratio"] = float(cl.sum()) / max(float((cl ** 2).sum()), 1.0)

    # ---- replicated small tensors (weights/features scaled for fp8) ----
    def arrange(mT, cols, scale):
        n_chunks = mT.shape[0] // D
        a = mT.reshape(n_chunks, D, cols).transpose(1, 0, 2).reshape(D, -1)
        return np.ascontiguousarray((a * scale).astype(fp8))

    wsT = arrange(Ws.T, D, WSCALE)
    wtT = arrange(Wt.T, D, WSCALE)
    fsT = arrange(f_s.T, B, 1.0)
    ftT = arrange(f_t.T, B, 1.0)
    pos1T = np.ascontiguousarray(memory_v1[idx].T)
    pos2T = np.ascontiguousarray(memory_v2[idx].T)
    ident = np.eye(D, dtype=np.float32).astype(fp8)
    emb8 = np.concatenate([wsT, fsT, wtT, ftT, ident], axis=1)
    assert emb8.shape == (D, 4736), emb8.shape

    # host mirror of the device embed (same fp8-quantized operands the PE
    # sees, so rinv matches the device vraw to fp32-accumulation noise)
    def host_rinv(wq, fq, b):
        vraw = (fq.astype(np.float32) @ wq.astype(np.float32).T) / WSCALE + b
        return (1.0 / np.sqrt((vraw ** 2).sum(axis=1))).astype(np.float32)

    rinv_s = host_rinv((Ws * WSCALE).astype(fp8), f_s.astype(fp8), bs)
    rinv_t = host_rinv((Wt * WSCALE).astype(fp8), f_t.astype(fp8), bt)
    esc_s = rinv_s.astype(np.float64) / (NCE_T * BANK_SCALE)
    esc_t = rinv_t.astype(np.float64) / (NCE_T * BANK_SCALE)
    escv = np.concatenate([esc_s, esc_t]).astype(np.float32).reshape(D, 1)
    fpk = np.zeros((D, 259), dtype=np.float32)
    fpk[:, 0] = bs
    fpk[:, 1] = bt
    fpk[:, 2:66] = pos1T
    fpk[:, 66:130] = pos2T
    fpk[:, 130:194] = np.tile(rinv_s.reshape(1, B), (D, 1))
    fpk[:, 194:258] = np.tile(rinv_t.reshape(1, B), (D, 1))
    fpk[:, 258:259] = escv

    # L-windows: L[row, n] = ln(cnt)/escale_row (cnt=0 -> -LCLAMP);
    # C-windows: the raw count (exact small ints in fp8)
    with np.errstate(divide="ignore"):
        lncnt = np.log(counts)          # [B, N_DATA], -inf where cnt=0
    L_s = lncnt / esc_s[:, None]
    L_t = lncnt / esc_t[:, None]
    L = np.concatenate([L_s, L_t], axis=0)      # [128, N_DATA]
    L = np.clip(L, -LCLAMP, LCLAMP)
    L8 = np.full((D, N_PAD), -LCLAMP, dtype=np.float32)
    L8[:, :N_DATA] = L
    cnt2 = np.zeros((D, N_PAD), dtype=np.float32)
    cnt2[:B, :N_DATA] = counts
    cnt2[B:, :N_DATA] = counts
    L8[:, ~lmask_n] = cnt2[:, ~lmask_n]
    L8 = L8.astype(fp8)

    # pad the n dimension to N_PAD (zero rows: L=-240 kills them)
    def pad_cols(a):
        out = np.zeros((a.shape[0], N_PAD), dtype=fp8)
        out[:, :N_DATA] = a
        return out

    m1s = pad_cols((memory_v1.T * BANK_SCALE).astype(fp8))
    m2s = pad_cols((memory_v2.T * BANK_SCALE).astype(fp8))

    in_maps = []
    for c in range(N_CORES):
        sl = slice(c * R, (c + 1) * R)
        # fused interleaved banks [D, N_WIN, 2, W]: k=0 -> mem2 (s side),
        # k=1 -> mem1 (t side)
        mf = np.empty((D, N_WIN, 2, W), dtype=fp8)
        mf[:, :, 0, :] = m2s[:, sl].reshape(D, N_WIN, W)
        mf[:, :, 1, :] = m1s[:, sl].reshape(D, N_WIN, W)
        in_maps.append({
            "emb8": emb8, "fpk": fpk,
            "memf": np.ascontiguousarray(mf.reshape(D, 2 * R)),
            "lnc": np.ascontiguousarray(L8[:, sl]),
        })
    return in_maps


def _combine(out_accs):
    """out_accs: per-core [128, 8] float arrays -> scalar loss (float32)."""
    outs = [np.asarray(o).astype(np.float64) for o in out_accs]
    m2_ratio = _CACHE["m2_ratio"]

    def side_loss(half, possum):
        # M1 = sum cnt*e ; M2 = stride * (ratio*sum(cnt*e)^2 + sum cnt*e^2)
        M = [sum(o[half, m].sum() for o in outs) for m in range(3)]
        M[1] = float(M2_STRIDE) * (m2_ratio * M[1] + M[2])
        Z = M[0] / (B * KP1) * N_DATA
        cz = CVAL * Z
        # sum cnt*ln(x+c) = B*KP1*ln(c) + sum_m (-1)^(m+1) M_m/(m cz^m)
        series = sum((-1.0) ** m * M[m] / ((m + 1) * cz ** (m + 1))
                     for m in range(2))
        sum_ln_xc = B * KP1 * np.log(CVAL) + series
        neg_b_loss = (possum / NCE_T - B * np.log(Z)
                      + B * NCE_K * np.log(NCE_K * PN) - sum_ln_xc)
        return -neg_b_loss / B

    s_loss = side_loss(slice(0, B), outs[0][:, 3].sum())
    t_loss = side_loss(slice(B, D), outs[0][:, 4].sum())
    return np.float32(s_loss + t_loss)


def kernel(f_s, f_t, idx, contrast_idx, Ws, bs, Wt, bt, memory_v1, memory_v2):
    in_maps = _prepare_in_maps(f_s, f_t, idx, contrast_idx, Ws, bs, Wt, bt,
                               memory_v1, memory_v2)
    if "nc" not in _CACHE:
        _CACHE["nc"] = _build_program()
    nc = _CACHE["nc"]
    res = run_bass_kernel_spmd(nc, in_maps, list(range(N_CORES)), trace=TRACE)
    _CACHE["last_results"] = res
    return kernel_combine_results(res)


def kernel_combine_results(res):
    return _combine([res.results[c]["out_acc"] for c in range(N_CORES)])
